# revision 9
# baseline (speedup 1.0000x reference)
"""Trainium2 Bass kernel for nn_Discriminator (MoE-routing discriminator).

Strategy (8 NeuronCores, single SPMD NEFF launch):
  Phase A (expert layer, column-parallel): every core reads ALL samples'
    inputs but only a 128-column slice of every expert's W_in. Core c
    computes hT_c = lrelu(W[:, c*128:(c+1)*128].T @ x.T + b) for all
    (bucketed-by-expert) samples -> [128 features, NT samples].
    W and x are shipped as fp8e4 (W pre-scaled x16, descale folded into
    the epilogue activation scale) and the matmuls run DoubleRow
    (256-deep contraction per pass): this halves both the HBM traffic
    (the dominant cost) and the PE time vs bf16.
  AllToAll x2 turns the feature-sharded hT into sample-sharded hT.
    Experts 0-5 finish within the first ~30% of phase A, so their
    AllToAll is staged (via the otherwise-idle GpSimd SWDGE, NOT the
    flow-controlled sync/scalar HW queues) and triggered (from the
    otherwise-idle Vector engine) early, completing inside phase A.
    Expert 6's AllToAll goes right after its epilogue. A tiny dummy
    AllToAll fires at t=0 to absorb the collective firmware's one-time
    wake-up latency off the critical path.
  Phase B (shared fc stack, data-parallel): each core runs the 3-layer
    fc stack for its sample shard, keeping activations transposed
    (features on partitions) so no transposes are ever needed. The
    experts-0-5 chunk of phase B hides inside phase A's expert-6 tail.

  Engine roles: sync = phase-A W stream + receives + output DMA;
  scalar = phase-A x stream + all activations; gpsimd = small consts +
  collective staging + fc weights; vector = collective triggers;
  tensor = matmuls + transposes.
"""
import os
import ml_dtypes
import numpy as np
from contextlib import ExitStack

import concourse.bacc as bacc
import concourse.bass as bass
import concourse.tile as tile
from concourse import mybir
from concourse.masks import make_identity
from concourse.tile_rust import add_dep_helper
from concourse.bass_utils import run_bass_kernel_spmd

P = 128
NCORES = 8
EMBED_DIM = 16
HIDDEN = 256
N_EXPERTS = 7
SIZES = [(2 ** (o + 1) + 1) ** 2 for o in range(N_EXPERTS)]  # 9..16641
S_MAX = SIZES[-1]
H4 = 4 * HIDDEN   # 1024
H2 = 2 * HIDDEN   # 512
H1 = HIDDEN       # 256
BF16 = mybir.dt.bfloat16
F32 = mybir.dt.float32
FP8 = mybir.dt.float8e4
NPFP8 = ml_dtypes.float8_e4m3fn
WSCALE = 16.0     # W_in pre-scale before fp8 cast; descaled in epilogue
GRP = 24          # k-tile PAIRS per DMA batch (48 k-tiles)

_CACHE = {}
last_run = None


def _round_up(x, m):
    return (x + m - 1) // m * m


def _part_major(a, ktiles, width):
    """[ktiles*128, width] -> [128, ktiles*width] partition-major layout."""
    return np.ascontiguousarray(
        a.reshape(ktiles, P, width).transpose(1, 0, 2).reshape(P, ktiles * width)
    )


def _ktiles2(s):
    """k-tiles for payload s+EMBED_DIM, rounded up to an even count."""
    return _round_up(_round_up(s + EMBED_DIM, P) // P, 2)


def build_program(n_pads):
    """Build the SPMD Bass program. n_pads: per-expert padded sample counts."""
    ktiles = [_ktiles2(s) for s in SIZES]   # even per expert
    pairs = [k // 2 for k in ktiles]
    T = sum(ktiles)
    NT1 = sum(n_pads[:-1])       # experts 0-5 columns (first AllToAll)
    NT2 = n_pads[-1]             # expert 6 columns (second AllToAll)
    G1, G2 = NT1 // NCORES, NT2 // NCORES

    nc = bacc.Bacc("TRN2", target_bir_lowering=False, debug=False,
                   num_devices=NCORES)

    # experts 0-5 x packed in ONE tensor (one DMA); expert 6 separate
    xsm_w = sum(ktiles[o] * n_pads[o] for o in range(N_EXPERTS - 1))
    xsm_p = nc.declare_dram_parameter("xsm", [P, xsm_w], FP8, isOutput=False)
    xt6_p = nc.declare_dram_parameter(
        "xt6", [P, ktiles[-1] * n_pads[-1]], FP8, isOutput=False)
    wt_p = nc.declare_dram_parameter("wt", [P, T * P], FP8, isOutput=False)
    w1_p = nc.declare_dram_parameter("w1", [P, 8 * H2], BF16, isOutput=False)
    w2_p = nc.declare_dram_parameter("w2", [P, 4 * H1], BF16, isOutput=False)
    w3_p = nc.declare_dram_parameter("w3", [P, 2 * 1], BF16, isOutput=False)
    bin_p = nc.declare_dram_parameter("bin", [P, N_EXPERTS], F32, isOutput=False)
    b1_p = nc.declare_dram_parameter("b1", [P, 4], F32, isOutput=False)
    b2_p = nc.declare_dram_parameter("b2", [P, 2], F32, isOutput=False)
    b3_p = nc.declare_dram_parameter("b3", [1, 1], F32, isOutput=False)
    out_p = nc.declare_dram_parameter("out", [1, G1 + G2], F32, isOutput=True)

    LR = mybir.ActivationFunctionType.Prelu
    SIG = mybir.ActivationFunctionType.Sigmoid
    CP = mybir.ActivationFunctionType.Copy
    DR = mybir.MatmulPerfMode.DoubleRow
    INV = 1.0 / WSCALE

    with tile.TileContext(nc) as tc, ExitStack() as ctx:
        wpool = ctx.enter_context(tc.tile_pool(name="wpool", bufs=3))
        xpool = ctx.enter_context(tc.tile_pool(name="xpool", bufs=3))
        hpool = ctx.enter_context(tc.tile_pool(name="hpool", bufs=1))
        cpool = ctx.enter_context(tc.tile_pool(name="cpool", bufs=1))
        pspool = ctx.enter_context(tc.tile_pool(name="pspool", bufs=1, space="PSUM"))
        drpool = ctx.enter_context(tc.tile_pool(name="drpool", bufs=1, space="DRAM"))

        # per-engine DMA issue-order chain for the two HW DGE queues
        qlast = {}

        def qdma(eng, out, in_):
            h = eng.dma_start(out, in_)
            key = id(eng)
            if key in qlast:
                add_dep_helper(h.ins, qlast[key].ins, sync=False,
                               reason="hw queue order")
            qlast[key] = h
            return h

        # explicit issue-order chain for the gpsimd SWDGE stream: without it
        # the Tile scheduler may park an early collective trigger behind a
        # later staging DMA's semaphore wait
        gchain = [None]

        def glink(h):
            if gchain[0] is not None:
                add_dep_helper(h.ins, gchain[0].ins, sync=False,
                               reason="gpsimd order")
            gchain[0] = h
            return h

        def gdma(out, in_):
            return glink(nc.gpsimd.dma_start(out, in_))

        def gcc(ins, outs):
            return glink(nc.gpsimd.collective_compute(
                "AllToAll", mybir.AluOpType.bypass, ins=ins, outs=outs,
                replica_groups=[list(range(NCORES))]))

        # dummy AllToAll FIRST: wakes the collective firmware (~10us, one
        # time) while phase A streams; later collectives get a warm CC core
        dum_sb = cpool.tile([NCORES, 16], F32)
        glink(nc.gpsimd.memset(dum_sb[:], 0.0))
        dum_in = drpool.tile([NCORES, 16], F32, name="dum_in")
        dum_out = drpool.tile([NCORES, 16], F32, name="dum_out")
        gdma(dum_in[:], dum_sb[:])
        gcc([dum_in[:]], [dum_out[:]])

        # ---- small constants, off the HW queues ----
        binsb = cpool.tile([P, N_EXPERTS], F32)
        gdma(binsb[:], bin_p[:])
        b1sb = cpool.tile([P, 4], F32)
        gdma(b1sb[:], b1_p[:])
        b2sb = cpool.tile([P, 2], F32)
        gdma(b2sb[:], b2_p[:])
        b3sb = cpool.tile([1, 1], F32)
        gdma(b3sb[:], b3_p[:])

        ident = cpool.tile([P, P], BF16)
        make_identity(nc, ident[:])
        # dummy sigmoid: preload the act table set containing sigmoid+prelu
        dummy = cpool.tile([1, 1], F32)
        nc.scalar.activation(dummy[:], b3sb[:], SIG)
        # fc weights (declared here, DMA'd on gpsimd after the first
        # AllToAll is staged+triggered: done by ~18us, needed at ~25us)
        w1sb = cpool.tile([P, 8 * H2], BF16)
        w2sb = cpool.tile([P, 4 * H1], BF16)
        w3sb = cpool.tile([P, 2], BF16)

        # ---------------- Phase A: expert layer (column slice) ----------------
        # ALL phase-A DMAs are issued up front on the two HW queues, with no
        # compute instructions interleaved on the issuing engines: the queues
        # stream back-to-back at full HBM rate while tensor chases the data.
        H1t = hpool.tile([P, NT1], BF16)
        H2t = hpool.tile([P, NT2], BF16)

        Tsm = sum(ktiles[:-1])      # 54 k-tiles for experts 0-5
        wsm = cpool.tile([P, Tsm * P], FP8)
        xsm = cpool.tile([P, xsm_w], FP8)
        qdma(nc.sync, wsm[:], wt_p[:, :Tsm * P])
        qdma(nc.scalar, xsm[:], xsm_p[:])

        # expert-6 groups: small enough SBUF footprint to keep all resident
        pr6, npad6 = pairs[-1], n_pads[-1]
        g6 = []
        g0 = 0
        while g0 < pr6:
            g6.append((g0, min(GRP, pr6 - g0)))
            g0 += GRP
        w6t, x6t = [], []
        for gi, (g0, gcnt) in enumerate(g6):
            weng = nc.sync if gi % 2 == 0 else nc.scalar
            xeng = nc.scalar if gi % 2 == 0 else nc.sync
            wg = cpool.tile([P, gcnt * 2 * P], FP8, name=f"w6_{gi}")
            qdma(weng, wg[:],
                 wt_p[:, (Tsm + 2 * g0) * P:(Tsm + 2 * (g0 + gcnt)) * P])
            xg = cpool.tile([P, gcnt * 2 * npad6], FP8, name=f"x6_{gi}")
            qdma(xeng, xg[:],
                 xt6_p[:, 2 * g0 * npad6:2 * (g0 + gcnt) * npad6])
            w6t.append(wg)
            x6t.append(xg)

        # ---- compute: experts 0-5 (slicing the batched wsm/xsm tiles) ------
        base_tw = np.cumsum([0] + ktiles[:-1]).tolist()
        base_x = np.cumsum(
            [0] + [ktiles[o] * n_pads[o] for o in range(N_EXPERTS - 1)]).tolist()
        offs = np.cumsum([0] + n_pads[:-1]).tolist()
        for o in range(N_EXPERTS - 1):
            pr, npad = pairs[o], n_pads[o]
            ps = pspool.tile([P, npad], F32, tag="psA", padded_shape=[P, 512],
                             bufs=3, name=f"psA{o}")
            for j in range(pr):
                nc.tensor.matmul(
                    ps[:],
                    wsm[:, (base_tw[o] + 2 * j) * P:(base_tw[o] + 2 * j + 2) * P
                        ].rearrange("p (two m) -> p two m", two=2),
                    xsm[:, base_x[o] + 2 * j * npad:base_x[o] + (2 * j + 2) * npad
                        ].rearrange("p (two n) -> p two n", two=2),
                    start=(j == 0), stop=(j == pr - 1), perf_mode=DR)
            nc.scalar.activation(H1t[:, offs[o]:offs[o] + npad], ps[:], LR,
                                 bias=binsb[:, o:o + 1], scale=INV, alpha=0.2)

        # -------- early AllToAll for experts 0-5, staged via gpsimd SWDGE ----
        a2a_in1 = drpool.tile([NCORES * P, G1], BF16, name="a2a_in1")
        a2a_out1 = drpool.tile([NCORES * P, G1], BF16, name="a2a_out1")
        gdma(a2a_in1[:].rearrange("(s p) j -> p s j", p=P),
             H1t[:].rearrange("p (s j) -> p s j", s=NCORES))
        gcc([a2a_in1[:]], [a2a_out1[:]])
        # fc weights now, on the same gpsimd SWDGE queue (behind the
        # time-critical staging+trigger, ahead of anything blocking)
        gdma(w1sb[:], w1_p[:])
        gdma(w2sb[:], w2_p[:])
        gdma(w3sb[:], w3_p[:])

        # ---- compute: expert 6 (73% of the k-tiles) ------------------------
        ps6 = pspool.tile([P, npad6], F32, tag="psA", padded_shape=[P, 512],
                          bufs=3, name="psA6")
        for gi, (g0, gcnt) in enumerate(g6):
            for j in range(gcnt):
                g = g0 + j
                nc.tensor.matmul(
                    ps6[:],
                    w6t[gi][:, 2 * j * P:(2 * j + 2) * P].rearrange(
                        "p (two m) -> p two m", two=2),
                    x6t[gi][:, 2 * j * npad6:(2 * j + 2) * npad6].rearrange(
                        "p (two n) -> p two n", two=2),
                    start=(g == 0), stop=(g == pr6 - 1), perf_mode=DR)
        nc.scalar.activation(H2t[:], ps6[:], LR,
                             bias=binsb[:, N_EXPERTS - 1:N_EXPERTS],
                             scale=INV, alpha=0.2)

        # hT1 receive: one batched 3D-AP load on the (now idle) sync queue
        hT1sb = hpool.tile([P, NCORES * G1], BF16)
        qdma(nc.sync,
             hT1sb[:].rearrange("p (s j) -> p s j", s=NCORES),
             a2a_out1[:].rearrange("(s p) j -> p s j", p=P))

        # transpose H2t to sample-major for the second AllToAll's wire format
        hs6 = []
        h2chunks = [(0, min(P, NT2))]
        if NT2 > P:
            h2chunks.append((P, NT2 - P))
        for i, (c0, cw) in enumerate(h2chunks):
            pst = pspool.tile([cw, P], BF16, tag="pstr", padded_shape=[P, P],
                              bufs=2, name=f"pstr{i}")
            nc.tensor.transpose(pst[:], H2t[:, c0:c0 + cw], ident[:])
            t = hpool.tile([cw, P], BF16, tag=f"hs6_{i}", name=f"hs6_{i}")
            nc.scalar.activation(t[:], pst[:], CP)
            hs6.append(t)

        a2a_in2 = drpool.tile([NT2, P], BF16, name="a2a_in2")
        a2a_out2 = drpool.tile([NT2, P], BF16, name="a2a_out2")
        for i, (c0, cw) in enumerate(h2chunks):
            gdma(a2a_in2[c0:c0 + cw, :], hs6[i][:])
        gcc([a2a_in2[:]], [a2a_out2[:]])

        # receive: batched loads (as many ranks as fit in 128 partitions),
        # one PE transpose per batch; the transposed psum's free axis is
        # (rank, sample) row-major which IS hT2's column order
        hT2 = hpool.tile([P, NCORES * G2], BF16)
        rb = max(1, P // G2)          # ranks per batch
        r0 = 0
        bi = 0
        while r0 < NCORES:
            rcnt = min(rb, NCORES - r0)
            rows = rcnt * G2
            st2 = hpool.tile([rows, P], BF16, tag="st2", bufs=2, name=f"st2_{bi}")
            qdma(nc.sync, st2[:], a2a_out2[r0 * G2:(r0 + rcnt) * G2, :])
            pst = pspool.tile([P, rows], BF16, tag="pstr", padded_shape=[P, P],
                              bufs=2, name=f"pst2_{bi}")
            nc.tensor.transpose(pst[:], st2[:], ident[:rows, :rows])
            nc.scalar.activation(hT2[:, r0 * G2:(r0 + rcnt) * G2], pst[:], CP)
            r0 += rcnt
            bi += 1

        # ---------------- Phase B: fc stack on my sample shard ---------------
        # chunk 1 (experts 0-5 samples) overlaps phase A's expert-6 tail;
        # chunk 2 (expert 6 samples) runs after the second AllToAll.
        def fc_chunk(gs, rhs_of, ocol):
            z1 = hpool.tile([P, 4 * gs], BF16, tag=f"z1_{ocol}", name=f"z1_{ocol}")
            for m in range(4):
                ps1 = pspool.tile([P, gs], F32, tag="psB",
                                  padded_shape=[P, 512], bufs=2, name=f"ps1_{ocol}_{m}")
                for r in range(NCORES):
                    nc.tensor.matmul(
                        ps1[:], w1sb[:, r * H2 + m * P:r * H2 + (m + 1) * P],
                        rhs_of(r),
                        start=(r == 0), stop=(r == NCORES - 1))
                nc.scalar.activation(z1[:, m * gs:(m + 1) * gs], ps1[:], LR,
                                     bias=b1sb[:, m:m + 1], alpha=0.2)

            z2 = hpool.tile([P, 2 * gs], BF16, tag=f"z2_{ocol}", name=f"z2_{ocol}")
            for m in range(2):
                ps2 = pspool.tile([P, gs], F32, tag="psB",
                                  padded_shape=[P, 512], bufs=2, name=f"ps2_{ocol}_{m}")
                for r in range(4):
                    nc.tensor.matmul(
                        ps2[:], w2sb[:, r * H1 + m * P:r * H1 + (m + 1) * P],
                        z1[:, r * gs:(r + 1) * gs],
                        start=(r == 0), stop=(r == 3))
                nc.scalar.activation(z2[:, m * gs:(m + 1) * gs], ps2[:], LR,
                                     bias=b2sb[:, m:m + 1], alpha=0.2)

            ps3 = pspool.tile([1, gs], F32, tag="psC", bufs=1, name=f"ps3_{ocol}")
            for r in range(2):
                nc.tensor.matmul(ps3[:], w3sb[:, r:r + 1],
                                 z2[:, r * gs:(r + 1) * gs],
                                 start=(r == 0), stop=(r == 1))
            osb = hpool.tile([1, gs], F32, tag=f"osb{ocol}", name=f"osb{ocol}")
            nc.scalar.activation(osb[:], ps3[:], SIG, bias=b3sb[:, 0:1])
            qdma(nc.sync, out_p[:, ocol:ocol + gs], osb[:])

        fc_chunk(G1, lambda r: hT1sb[:, r * G1:(r + 1) * G1], 0)
        fc_chunk(G2, lambda r: hT2[:, r * G2:(r + 1) * G2], G1)

    nc.compile()
    return nc


def kernel(mazes, orders, embed_table, W_in, b_in, W1, b1, W2, b2, W3, b3):
    mazes = np.asarray(mazes)
    orders = np.asarray(orders)
    B = mazes.shape[0]

    # ---- sample routing (host) ----
    idx = [np.where(orders == o)[0] for o in range(N_EXPERTS)]
    ns = [len(i) for i in idx]
    n_pads = [max(16, _round_up(n, 16)) for n in ns]
    G1 = sum(n_pads[:-1]) // NCORES
    G2 = n_pads[-1] // NCORES
    ktiles = [_ktiles2(s) for s in SIZES]
    T = sum(ktiles)

    # ---- per-expert xT buffers (fp8, shared across cores) ----
    emb8 = np.asarray(embed_table, NPFP8)
    xparts = []
    for o in range(N_EXPERTS):
        s, kt, npad = SIZES[o], ktiles[o], n_pads[o]
        X = np.zeros((kt * P, npad), NPFP8)
        X[:s, :ns[o]] = np.asarray(mazes[idx[o], :s], NPFP8).T
        X[s:s + EMBED_DIM, :ns[o]] = emb8[o][:, None]
        xparts.append(_part_major(X, kt, npad))
    xts = {"xsm": np.ascontiguousarray(np.concatenate(xparts[:-1], axis=1)),
           "xt6": xparts[-1]}

    # ---- per-core W_in column slices (scaled x16, fp8) ----
    W_in = np.asarray(W_in)
    w8 = []
    for o in range(N_EXPERTS):
        s, kt = SIZES[o], ktiles[o]
        Wo = np.zeros((kt * P, H4), NPFP8)
        Wo[:s] = np.clip(W_in[o, :s] * WSCALE, -240, 240).astype(NPFP8)
        Wo[s:s + EMBED_DIM] = np.clip(
            W_in[o, S_MAX:] * WSCALE, -240, 240).astype(NPFP8)
        w8.append(Wo)
    wts = []
    for c in range(NCORES):
        Wc = np.concatenate([w[:, c * P:(c + 1) * P] for w in w8], axis=0)
        wts.append(_part_major(Wc, T, P))

    # ---- shared fc stack ----
    W1_16 = _part_major(np.asarray(W1, ml_dtypes.bfloat16), 8, H2)
    W2_16 = _part_major(np.asarray(W2, ml_dtypes.bfloat16), 4, H1)
    W3_16 = _part_major(np.asarray(W3, ml_dtypes.bfloat16), 2, 1)
    b1t = np.ascontiguousarray(np.asarray(b1, np.float32).reshape(4, P).T)
    b2t = np.ascontiguousarray(np.asarray(b2, np.float32).reshape(2, P).T)
    b3t = np.asarray(b3, np.float32).reshape(1, 1)
    b_in = np.asarray(b_in, np.float32)

    key = (tuple(n_pads), GRP)
    if key not in _CACHE:
        _CACHE[key] = build_program(n_pads)
    nc = _CACHE[key]

    in_maps = []
    for c in range(NCORES):
        m = dict(xts)
        m["wt"] = wts[c]
        m["w1"], m["w2"], m["w3"] = W1_16, W2_16, W3_16
        m["bin"] = np.ascontiguousarray(
            np.stack([b_in[o, c * P:(c + 1) * P] for o in range(N_EXPERTS)], 1))
        m["b1"], m["b2"], m["b3"] = b1t, b2t, b3t
        in_maps.append(m)

    trace = os.environ.get("KERNEL_TRACE") == "1"
    res = run_bass_kernel_spmd(nc, in_maps, list(range(NCORES)), trace=trace)
    global last_run
    last_run = res

    allc = np.stack([res.results[c]["out"][0] for c in range(NCORES)])  # [8, G1+G2]
    half1 = allc[:, :G1].reshape(-1)   # experts 0-5 padded samples
    half2 = allc[:, G1:].reshape(-1)   # expert 6 padded samples

    full = np.zeros((B, 1), np.float32)
    offs = np.cumsum([0] + n_pads[:-2])
    for o in range(N_EXPERTS - 1):
        full[idx[o], 0] = half1[offs[o]:offs[o] + ns[o]]
    full[idx[N_EXPERTS - 1], 0] = half2[:ns[N_EXPERTS - 1]]
    return full


# revision 13
# speedup vs baseline: 1.4248x; 1.4248x over previous
"""Trainium2 Bass kernel for nn_Discriminator (MoE-routing discriminator).

Strategy (8 NeuronCores, single SPMD NEFF launch):
  Phase A (expert layer, column-parallel): every core reads ALL samples'
    inputs but only a 128-column slice of every expert's W_in. Core c
    computes hT_c = lrelu(W[:, c*128:(c+1)*128].T @ x.T + b) for all
    (bucketed-by-expert) samples -> [128 features, NT samples].
    W and x are shipped as fp8e4 (W pre-scaled x16, descale folded into
    the epilogue activation scale) and the matmuls run DoubleRow
    (256-deep contraction per pass): this halves both the HBM traffic
    (the dominant cost) and the PE time vs bf16.
  AllToAll x2 turns the feature-sharded hT into sample-sharded hT.
    Experts 0-5 finish within the first ~30% of phase A, so their
    AllToAll is staged (via the otherwise-idle GpSimd SWDGE, NOT the
    flow-controlled sync/scalar HW queues) and triggered (from the
    otherwise-idle Vector engine) early, completing inside phase A.
    Expert 6's AllToAll goes right after its epilogue. A tiny dummy
    AllToAll fires at t=0 to absorb the collective firmware's one-time
    wake-up latency off the critical path.
  Phase B (shared fc stack, data-parallel): each core runs the 3-layer
    fc stack for its sample shard, keeping activations transposed
    (features on partitions) so no transposes are ever needed. The
    experts-0-5 chunk of phase B hides inside phase A's expert-6 tail.

  Engine roles: sync = phase-A W stream + receives + output DMA;
  scalar = phase-A x stream + all activations; gpsimd = small consts +
  collective staging + fc weights; vector = collective triggers;
  tensor = matmuls + transposes.
"""
import os
import ml_dtypes
import numpy as np
from contextlib import ExitStack

import concourse.bacc as bacc
import concourse.bass as bass
import concourse.tile as tile
from concourse import mybir
from concourse.masks import make_identity
from concourse.tile_rust import add_dep_helper
from concourse.bass_utils import run_bass_kernel_spmd

P = 128
NCORES = 8
EMBED_DIM = 16
HIDDEN = 256
N_EXPERTS = 7
SIZES = [(2 ** (o + 1) + 1) ** 2 for o in range(N_EXPERTS)]  # 9..16641
S_MAX = SIZES[-1]
H4 = 4 * HIDDEN   # 1024
H2 = 2 * HIDDEN   # 512
H1 = HIDDEN       # 256
BF16 = mybir.dt.bfloat16
F32 = mybir.dt.float32
FP8 = mybir.dt.float8e4
NPFP8 = ml_dtypes.float8_e4m3fn
WSCALE = 16.0     # W_in pre-scale before fp8 cast; descaled in epilogue
GRP = 24          # k-tile PAIRS per DMA batch (48 k-tiles)

_CACHE = {}
last_run = None


def _round_up(x, m):
    return (x + m - 1) // m * m


def _part_major(a, ktiles, width):
    """[ktiles*128, width] -> [128, ktiles*width] partition-major layout."""
    return np.ascontiguousarray(
        a.reshape(ktiles, P, width).transpose(1, 0, 2).reshape(P, ktiles * width)
    )


def _ktiles2(s):
    """k-tiles for payload s+EMBED_DIM, rounded up to an even count."""
    return _round_up(_round_up(s + EMBED_DIM, P) // P, 2)


def build_program(n_pads):
    """Build the SPMD Bass program. n_pads: per-expert padded sample counts."""
    ktiles = [_ktiles2(s) for s in SIZES]   # even per expert
    pairs = [k // 2 for k in ktiles]
    T = sum(ktiles)
    NT1 = sum(n_pads[:-1])       # experts 0-5 columns (first AllToAll)
    NT2 = n_pads[-1]             # expert 6 columns (second AllToAll)
    G1, G2 = NT1 // NCORES, NT2 // NCORES

    nc = bacc.Bacc("TRN2", target_bir_lowering=False, debug=False,
                   num_devices=NCORES)

    # experts 0-5 x packed in ONE tensor (one DMA); expert 6 separate
    xsm_w = sum(ktiles[o] * n_pads[o] for o in range(N_EXPERTS - 1))
    xsm_p = nc.declare_dram_parameter("xsm", [P, xsm_w], FP8, isOutput=False)
    xt6_p = nc.declare_dram_parameter(
        "xt6", [P, ktiles[-1] * n_pads[-1]], FP8, isOutput=False)
    wt_p = nc.declare_dram_parameter("wt", [P, T * P], FP8, isOutput=False)
    w1_p = nc.declare_dram_parameter("w1", [P, 8 * H2], BF16, isOutput=False)
    w2_p = nc.declare_dram_parameter("w2", [P, 4 * H1], BF16, isOutput=False)
    w3_p = nc.declare_dram_parameter("w3", [P, 2 * 1], BF16, isOutput=False)
    bin_p = nc.declare_dram_parameter("bin", [P, N_EXPERTS], F32, isOutput=False)
    b1_p = nc.declare_dram_parameter("b1", [P, 4], F32, isOutput=False)
    b2_p = nc.declare_dram_parameter("b2", [P, 2], F32, isOutput=False)
    b3_p = nc.declare_dram_parameter("b3", [1, 1], F32, isOutput=False)
    out_p = nc.declare_dram_parameter("out", [1, G1 + G2], F32, isOutput=True)

    LR = mybir.ActivationFunctionType.Prelu
    SIG = mybir.ActivationFunctionType.Sigmoid
    CP = mybir.ActivationFunctionType.Copy
    DR = mybir.MatmulPerfMode.DoubleRow
    INV = 1.0 / WSCALE

    with tile.TileContext(nc) as tc, ExitStack() as ctx:
        wpool = ctx.enter_context(tc.tile_pool(name="wpool", bufs=3))
        xpool = ctx.enter_context(tc.tile_pool(name="xpool", bufs=3))
        hpool = ctx.enter_context(tc.tile_pool(name="hpool", bufs=1))
        cpool = ctx.enter_context(tc.tile_pool(name="cpool", bufs=1))
        pspool = ctx.enter_context(tc.tile_pool(name="pspool", bufs=1, space="PSUM"))
        drpool = ctx.enter_context(tc.tile_pool(name="drpool", bufs=1, space="DRAM"))

        # per-engine DMA issue-order chain for the two HW DGE queues
        qlast = {}

        def qdma(eng, out, in_):
            h = eng.dma_start(out, in_)
            key = id(eng)
            if key in qlast:
                add_dep_helper(h.ins, qlast[key].ins, sync=False,
                               reason="hw queue order")
            qlast[key] = h
            return h

        # explicit issue-order chain for the gpsimd SWDGE stream: without it
        # the Tile scheduler may park an early collective trigger behind a
        # later staging DMA's semaphore wait
        gchain = [None]

        def glink(h):
            if gchain[0] is not None:
                add_dep_helper(h.ins, gchain[0].ins, sync=False,
                               reason="gpsimd order")
            gchain[0] = h
            return h

        def gdma(out, in_):
            return glink(nc.gpsimd.dma_start(out, in_))

        def gcc(ins, outs):
            return glink(nc.gpsimd.collective_compute(
                "AllToAll", mybir.AluOpType.bypass, ins=ins, outs=outs,
                replica_groups=[list(range(NCORES))]))

        # dummy AllToAll FIRST: wakes the collective firmware (~10us, one
        # time) while phase A streams; later collectives get a warm CC core
        dum_sb = cpool.tile([NCORES, 16], F32)
        glink(nc.gpsimd.memset(dum_sb[:], 0.0))
        dum_in = drpool.tile([NCORES, 16], F32, name="dum_in")
        dum_out = drpool.tile([NCORES, 16], F32, name="dum_out")
        gdma(dum_in[:], dum_sb[:])
        gcc([dum_in[:]], [dum_out[:]])

        # ---- small constants, off the HW queues ----
        binsb = cpool.tile([P, N_EXPERTS], F32)
        gdma(binsb[:], bin_p[:])
        b1sb = cpool.tile([P, 4], F32)
        gdma(b1sb[:], b1_p[:])
        b2sb = cpool.tile([P, 2], F32)
        gdma(b2sb[:], b2_p[:])
        b3sb = cpool.tile([1, 1], F32)
        gdma(b3sb[:], b3_p[:])

        ident = cpool.tile([P, P], BF16)
        make_identity(nc, ident[:])
        # dummy sigmoid: preload the act table set containing sigmoid+prelu
        dummy = cpool.tile([1, 1], F32)
        nc.scalar.activation(dummy[:], b3sb[:], SIG)
        # fc weights on the gpsimd SWDGE queue: done by ~15us, needed ~30us
        w1sb = cpool.tile([P, 8 * H2], BF16)
        gdma(w1sb[:], w1_p[:])
        w2sb = cpool.tile([P, 4 * H1], BF16)
        gdma(w2sb[:], w2_p[:])
        w3sb = cpool.tile([P, 2], BF16)
        gdma(w3sb[:], w3_p[:])

        # ---------------- Phase A: expert layer (column slice) ----------------
        # ALL phase-A DMAs are issued up front on the two HW queues, with no
        # compute instructions interleaved on the issuing engines: the queues
        # stream back-to-back at full HBM rate while tensor chases the data.
        H1t = hpool.tile([P, NT1], BF16)
        H2t = hpool.tile([P, NT2], BF16)

        Tsm = sum(ktiles[:-1])      # 54 k-tiles for experts 0-5
        wsm = cpool.tile([P, Tsm * P], FP8)
        xsm = cpool.tile([P, xsm_w], FP8)
        qdma(nc.sync, wsm[:], wt_p[:, :Tsm * P])
        qdma(nc.scalar, xsm[:], xsm_p[:])

        # expert-6 groups: small enough SBUF footprint to keep all resident;
        # decreasing sizes so the final matmul trail after the last DMA is short
        pr6, npad6 = pairs[-1], n_pads[-1]
        g6 = []
        g0 = 0
        rem = pr6
        while rem > 0:
            gcnt = min(GRP, rem)
            if rem <= GRP and rem > GRP // 2:
                gcnt = (rem + 1) // 2   # split the tail into two smaller chunks
            g6.append((g0, gcnt))
            g0 += gcnt
            rem -= gcnt
        w6t, x6t = [], []
        for gi, (g0, gcnt) in enumerate(g6):
            weng = nc.sync if gi % 2 == 0 else nc.scalar
            xeng = nc.scalar if gi % 2 == 0 else nc.sync
            wg = cpool.tile([P, gcnt * 2 * P], FP8, name=f"w6_{gi}")
            qdma(weng, wg[:],
                 wt_p[:, (Tsm + 2 * g0) * P:(Tsm + 2 * (g0 + gcnt)) * P])
            xg = cpool.tile([P, gcnt * 2 * npad6], FP8, name=f"x6_{gi}")
            qdma(xeng, xg[:],
                 xt6_p[:, 2 * g0 * npad6:2 * (g0 + gcnt) * npad6])
            w6t.append(wg)
            x6t.append(xg)

        # ---- compute: experts 0-5 (slicing the batched wsm/xsm tiles) ------
        base_tw = np.cumsum([0] + ktiles[:-1]).tolist()
        base_x = np.cumsum(
            [0] + [ktiles[o] * n_pads[o] for o in range(N_EXPERTS - 1)]).tolist()
        offs = np.cumsum([0] + n_pads[:-1]).tolist()
        for o in range(N_EXPERTS - 1):
            pr, npad = pairs[o], n_pads[o]
            ps = pspool.tile([P, npad], F32, tag="psA", padded_shape=[P, 512],
                             bufs=3, name=f"psA{o}")
            for j in range(pr):
                nc.tensor.matmul(
                    ps[:],
                    wsm[:, (base_tw[o] + 2 * j) * P:(base_tw[o] + 2 * j + 2) * P
                        ].rearrange("p (two m) -> p two m", two=2),
                    xsm[:, base_x[o] + 2 * j * npad:base_x[o] + (2 * j + 2) * npad
                        ].rearrange("p (two n) -> p two n", two=2),
                    start=(j == 0), stop=(j == pr - 1), perf_mode=DR)
            nc.scalar.activation(H1t[:, offs[o]:offs[o] + npad], ps[:], LR,
                                 bias=binsb[:, o:o + 1], scale=INV, alpha=0.2)

        # -------- early AllToAll for experts 0-5 --------
        # staged on the sync HWDGE queue (idle once phase-A issue is done;
        # fast descriptor generation), triggered from the gpsimd chain
        a2a_in1 = drpool.tile([NCORES * P, G1], BF16, name="a2a_in1")
        a2a_out1 = drpool.tile([NCORES * P, G1], BF16, name="a2a_out1")
        qdma(nc.sync,
             a2a_in1[:].rearrange("(s p) j -> p s j", p=P),
             H1t[:].rearrange("p (s j) -> p s j", s=NCORES))
        gcc([a2a_in1[:]], [a2a_out1[:]])

        # ---- compute: expert 6 (73% of the k-tiles) ------------------------
        ps6 = pspool.tile([P, npad6], F32, tag="psA", padded_shape=[P, 512],
                          bufs=3, name="psA6")
        for gi, (g0, gcnt) in enumerate(g6):
            for j in range(gcnt):
                g = g0 + j
                nc.tensor.matmul(
                    ps6[:],
                    w6t[gi][:, 2 * j * P:(2 * j + 2) * P].rearrange(
                        "p (two m) -> p two m", two=2),
                    x6t[gi][:, 2 * j * npad6:(2 * j + 2) * npad6].rearrange(
                        "p (two n) -> p two n", two=2),
                    start=(g == 0), stop=(g == pr6 - 1), perf_mode=DR)
        nc.scalar.activation(H2t[:], ps6[:], LR,
                             bias=binsb[:, N_EXPERTS - 1:N_EXPERTS],
                             scale=INV, alpha=0.2)

        # transpose H2t to sample-major for the second AllToAll's wire format
        hs6 = []
        h2chunks = [(0, min(P, NT2))]
        if NT2 > P:
            h2chunks.append((P, NT2 - P))
        for i, (c0, cw) in enumerate(h2chunks):
            pst = pspool.tile([cw, P], BF16, tag="pstr", padded_shape=[P, P],
                              bufs=2, name=f"pstr{i}")
            nc.tensor.transpose(pst[:], H2t[:, c0:c0 + cw], ident[:])
            t = hpool.tile([cw, P], BF16, tag=f"hs6_{i}", name=f"hs6_{i}")
            nc.scalar.activation(t[:], pst[:], CP)
            hs6.append(t)

        # stage AllToAll#2 on sync HWDGE (time-critical: gates trigger2),
        # BEFORE the mesh-gated hT1 receive in the queue order
        a2a_in2 = drpool.tile([NT2, P], BF16, name="a2a_in2")
        a2a_out2 = drpool.tile([NT2, P], BF16, name="a2a_out2")
        for i, (c0, cw) in enumerate(h2chunks):
            qdma(nc.sync, a2a_in2[c0:c0 + cw, :], hs6[i][:])
        gcc([a2a_in2[:]], [a2a_out2[:]])

        # hT1 receive: one batched 3D-AP load on the sync queue
        hT1sb = hpool.tile([P, NCORES * G1], BF16)
        qdma(nc.sync,
             hT1sb[:].rearrange("p (s j) -> p s j", s=NCORES),
             a2a_out1[:].rearrange("(s p) j -> p s j", p=P))

        # receive: batched loads (as many ranks as fit in 128 partitions),
        # one PE transpose per batch; the transposed psum's free axis is
        # (rank, sample) row-major which IS hT2's column order
        hT2 = hpool.tile([P, NCORES * G2], BF16)
        rb = max(1, P // G2)          # ranks per batch
        r0 = 0
        bi = 0
        while r0 < NCORES:
            rcnt = min(rb, NCORES - r0)
            rows = rcnt * G2
            st2 = hpool.tile([rows, P], BF16, tag="st2", bufs=2, name=f"st2_{bi}")
            qdma(nc.sync, st2[:], a2a_out2[r0 * G2:(r0 + rcnt) * G2, :])
            pst = pspool.tile([P, rows], BF16, tag="pstr", padded_shape=[P, P],
                              bufs=2, name=f"pst2_{bi}")
            nc.tensor.transpose(pst[:], st2[:], ident[:rows, :rows])
            nc.scalar.activation(hT2[:, r0 * G2:(r0 + rcnt) * G2], pst[:], CP)
            r0 += rcnt
            bi += 1

        # ---------------- Phase B: fc stack on my sample shard ---------------
        # chunk 1 (experts 0-5 samples) overlaps phase A's expert-6 tail;
        # chunk 2 (expert 6 samples) runs after the second AllToAll.
        def fc_chunk(gs, rhs_of, ocol):
            z1 = hpool.tile([P, 4 * gs], BF16, tag=f"z1_{ocol}", name=f"z1_{ocol}")
            for m in range(4):
                ps1 = pspool.tile([P, gs], F32, tag="psB",
                                  padded_shape=[P, 512], bufs=2, name=f"ps1_{ocol}_{m}")
                for r in range(NCORES):
                    nc.tensor.matmul(
                        ps1[:], w1sb[:, r * H2 + m * P:r * H2 + (m + 1) * P],
                        rhs_of(r),
                        start=(r == 0), stop=(r == NCORES - 1))
                nc.scalar.activation(z1[:, m * gs:(m + 1) * gs], ps1[:], LR,
                                     bias=b1sb[:, m:m + 1], alpha=0.2)

            z2 = hpool.tile([P, 2 * gs], BF16, tag=f"z2_{ocol}", name=f"z2_{ocol}")
            for m in range(2):
                ps2 = pspool.tile([P, gs], F32, tag="psB",
                                  padded_shape=[P, 512], bufs=2, name=f"ps2_{ocol}_{m}")
                for r in range(4):
                    nc.tensor.matmul(
                        ps2[:], w2sb[:, r * H1 + m * P:r * H1 + (m + 1) * P],
                        z1[:, r * gs:(r + 1) * gs],
                        start=(r == 0), stop=(r == 3))
                nc.scalar.activation(z2[:, m * gs:(m + 1) * gs], ps2[:], LR,
                                     bias=b2sb[:, m:m + 1], alpha=0.2)

            ps3 = pspool.tile([1, gs], F32, tag="psC", bufs=1, name=f"ps3_{ocol}")
            for r in range(2):
                nc.tensor.matmul(ps3[:], w3sb[:, r:r + 1],
                                 z2[:, r * gs:(r + 1) * gs],
                                 start=(r == 0), stop=(r == 1))
            osb = hpool.tile([1, gs], F32, tag=f"osb{ocol}", name=f"osb{ocol}")
            nc.scalar.activation(osb[:], ps3[:], SIG, bias=b3sb[:, 0:1])
            qdma(nc.sync, out_p[:, ocol:ocol + gs], osb[:])

        fc_chunk(G1, lambda r: hT1sb[:, r * G1:(r + 1) * G1], 0)
        fc_chunk(G2, lambda r: hT2[:, r * G2:(r + 1) * G2], G1)

    nc.compile()
    return nc


def kernel(mazes, orders, embed_table, W_in, b_in, W1, b1, W2, b2, W3, b3):
    mazes = np.asarray(mazes)
    orders = np.asarray(orders)
    B = mazes.shape[0]

    # ---- sample routing (host) ----
    idx = [np.where(orders == o)[0] for o in range(N_EXPERTS)]
    ns = [len(i) for i in idx]
    n_pads = [max(16, _round_up(n, 16)) for n in ns]
    G1 = sum(n_pads[:-1]) // NCORES
    G2 = n_pads[-1] // NCORES
    ktiles = [_ktiles2(s) for s in SIZES]
    T = sum(ktiles)

    # ---- per-expert xT buffers (fp8, shared across cores) ----
    emb8 = np.asarray(embed_table, NPFP8)
    xparts = []
    for o in range(N_EXPERTS):
        s, kt, npad = SIZES[o], ktiles[o], n_pads[o]
        X = np.zeros((kt * P, npad), NPFP8)
        X[:s, :ns[o]] = np.asarray(mazes[idx[o], :s], NPFP8).T
        X[s:s + EMBED_DIM, :ns[o]] = emb8[o][:, None]
        xparts.append(_part_major(X, kt, npad))
    xts = {"xsm": np.ascontiguousarray(np.concatenate(xparts[:-1], axis=1)),
           "xt6": xparts[-1]}

    # ---- per-core W_in column slices (scaled x16, fp8) ----
    W_in = np.asarray(W_in)
    w8 = []
    for o in range(N_EXPERTS):
        s, kt = SIZES[o], ktiles[o]
        Wo = np.zeros((kt * P, H4), NPFP8)
        Wo[:s] = np.clip(W_in[o, :s] * WSCALE, -240, 240).astype(NPFP8)
        Wo[s:s + EMBED_DIM] = np.clip(
            W_in[o, S_MAX:] * WSCALE, -240, 240).astype(NPFP8)
        w8.append(Wo)
    wts = []
    for c in range(NCORES):
        Wc = np.concatenate([w[:, c * P:(c + 1) * P] for w in w8], axis=0)
        wts.append(_part_major(Wc, T, P))

    # ---- shared fc stack ----
    W1_16 = _part_major(np.asarray(W1, ml_dtypes.bfloat16), 8, H2)
    W2_16 = _part_major(np.asarray(W2, ml_dtypes.bfloat16), 4, H1)
    W3_16 = _part_major(np.asarray(W3, ml_dtypes.bfloat16), 2, 1)
    b1t = np.ascontiguousarray(np.asarray(b1, np.float32).reshape(4, P).T)
    b2t = np.ascontiguousarray(np.asarray(b2, np.float32).reshape(2, P).T)
    b3t = np.asarray(b3, np.float32).reshape(1, 1)
    b_in = np.asarray(b_in, np.float32)

    key = (tuple(n_pads), GRP)
    if key not in _CACHE:
        _CACHE[key] = build_program(n_pads)
    nc = _CACHE[key]

    in_maps = []
    for c in range(NCORES):
        m = dict(xts)
        m["wt"] = wts[c]
        m["w1"], m["w2"], m["w3"] = W1_16, W2_16, W3_16
        m["bin"] = np.ascontiguousarray(
            np.stack([b_in[o, c * P:(c + 1) * P] for o in range(N_EXPERTS)], 1))
        m["b1"], m["b2"], m["b3"] = b1t, b2t, b3t
        in_maps.append(m)

    trace = os.environ.get("KERNEL_TRACE") == "1"
    res = run_bass_kernel_spmd(nc, in_maps, list(range(NCORES)), trace=trace)
    global last_run
    last_run = res

    allc = np.stack([res.results[c]["out"][0] for c in range(NCORES)])  # [8, G1+G2]
    half1 = allc[:, :G1].reshape(-1)   # experts 0-5 padded samples
    half2 = allc[:, G1:].reshape(-1)   # expert 6 padded samples

    full = np.zeros((B, 1), np.float32)
    offs = np.cumsum([0] + n_pads[:-2])
    for o in range(N_EXPERTS - 1):
        full[idx[o], 0] = half1[offs[o]:offs[o] + ns[o]]
    full[idx[N_EXPERTS - 1], 0] = half2[:ns[N_EXPERTS - 1]]
    return full
